# revision 1
# baseline (speedup 1.0000x reference)
"""Trainium2 8-core Bass kernel for nn_Attention_7112465842253.

Token-sharded attention: 512 tokens/core (cores 0-3 = batch 0, 4-7 = batch 1).
Per core: QKV projection in bf16 (q/k in transposed [chan, tok] layout, v in
natural [tok, chan]), RMSNorm via weighted-sumsq matmul + Ln/Exp rstd folded
into the RoPE tables, RoPE as x*C + (P@x)*S with a PE partition-swap matmul,
AllGather of K/V inside each 4-core batch group, non-causal attention in
scoresT layout (k-tokens on partitions; softmax denominator from a ones
column appended to V; Exp fused into the PSUM->SBUF eviction on ScalarE),
then the output projection. Host does layout prep and reassembly.
"""

import numpy as np

B, N, DIN, DIM, H, HD = 2, 2048, 1024, 1024, 16, 64
NCORE = 8
TOK = 512
EPS = 1e-6
BASE = 10000.0
KC = N // 128        # 16 k-token chunks
NHG = 4              # head groups of 4

_CACHE = {}


def _build_nc(dbg=None, single=False, zero_bias=False):
    import concourse.bass as bass
    import concourse.tile as tile
    from concourse import bacc, mybir
    from contextlib import ExitStack

    BF = mybir.dt.bfloat16
    F32 = mybir.dt.float32
    AF = mybir.ActivationFunctionType

    nc = bacc.Bacc(
        "TRN2", target_bir_lowering=False, debug=False,
        num_devices=(1 if single else NCORE),
    )

    # ---------------- DRAM parameters ----------------
    xT = nc.dram_tensor("xT", [DIN, TOK], BF, kind="ExternalInput")
    wqkv = nc.dram_tensor("wqkv", [DIN, 3 * DIM], BF, kind="ExternalInput")
    bqkv = nc.dram_tensor("bqkv", [1, 3 * DIM], BF, kind="ExternalInput")
    wsum = nc.dram_tensor("wsum", [DIN, 2], BF, kind="ExternalInput")
    swp = nc.dram_tensor("swp", [128, 128], BF, kind="ExternalInput")
    tabs = nc.dram_tensor("tabs", [4, 128, TOK], BF, kind="ExternalInput")
    wout = nc.dram_tensor("wout", [DIM, DIN], BF, kind="ExternalInput")
    out = nc.dram_tensor("out", [DIN, TOK], F32, kind="ExternalOutput")
    dbgt = (nc.dram_tensor("dbg", [DIN, TOK], F32, kind="ExternalOutput")
            if dbg else None)

    # internal DRAM
    agin = nc.dram_tensor("agin", [2048 * TOK], BF)   # khatT flat + v flat
    agout = nc.dram_tensor("agout", [4 * 2048 * TOK], BF)
    sescr = nc.dram_tensor("sescr", [1, 4 * TOK], F32)  # recip bcast scratch
    sescr2 = nc.dram_tensor("sescr2", [1, 4 * TOK], F32)  # sumexp staging

    RG = [[0, 1, 2, 3], [4, 5, 6, 7]]

    with tile.TileContext(nc) as tc, ExitStack() as CTX:
        # ---------------- persistent SBUF ----------------
        pp = CTX.enter_context(tc.tile_pool(name="persist", bufs=1))
        qhat = [pp.tile([128, TOK], BF, tag=f"qhat{c}", name=f"qhat{c}") for c in range(8)]
        ktf = [pp.tile([128, N], BF, tag=f"ktf{c}", name=f"ktf{c}") for c in range(8)]
        vaug = [pp.tile([128, 16 * 65], BF, tag=f"vaug{c}", name=f"vaug{c}") for c in range(KC)]
        attnT = [pp.tile([128, TOK], BF, tag=f"attnT{c}", name=f"attnT{c}") for c in range(8)]

        bias_sb = pp.tile([1, 3 * DIM], BF, tag="bias_sb", name="bias_sb")
        wsum_sb = pp.tile([128, 8, 2], BF, tag="wsum_sb", name="wsum_sb")
        swp_sb = pp.tile([128, 128], BF, tag="swp_sb", name="swp_sb")
        ones_t = pp.tile([1, TOK], BF, tag="ones_t", name="ones_t")
        ones_c = pp.tile([1, 128], BF, tag="ones_c", name="ones_c")
        eps_sb = pp.tile([1, 1], F32, tag="eps_sb", name="eps_sb")

        nc.vector.memset(ones_t[:], 1.0)
        nc.vector.memset(ones_c[:], 1.0)
        nc.vector.memset(eps_sb[:], EPS)

        # ---------------- phase 1: qkv + norm + rope + AG ----------------
        p1 = ExitStack()
        wq_pool = p1.enter_context(tc.tile_pool(name="wq", bufs=4))
        vw_pool = p1.enter_context(tc.tile_pool(name="vw", bufs=1))
        sq_pool = p1.enter_context(tc.tile_pool(name="sq", bufs=4))
        qtsb_pool = p1.enter_context(tc.tile_pool(name="qtsb", bufs=9))
        scr1_pool = p1.enter_context(tc.tile_pool(name="scr1", bufs=4))
        kvloc_pool = p1.enter_context(tc.tile_pool(name="kvloc", bufs=1))
        rstd_pool = p1.enter_context(tc.tile_pool(name="rstd", bufs=4))
        # PSUM budget (8 banks): qkvps 3 + swpp 3 + ssqp 1 + bcp 1
        qkvp = p1.enter_context(tc.tile_pool(name="qkvp", bufs=3, space="PSUM"))
        swpp = p1.enter_context(tc.tile_pool(name="swpp", bufs=3, space="PSUM"))
        ssqp = p1.enter_context(tc.tile_pool(name="ssqp", bufs=1, space="PSUM"))
        bcp = p1.enter_context(tc.tile_pool(name="bcp", bufs=1, space="PSUM"))
        misc1_pool = p1.enter_context(tc.tile_pool(name="misc1", bufs=1))
        xt_sb = misc1_pool.tile([128, 8, TOK], BF, tag="xt_sb", name="xt_sb")
        tab_r = [misc1_pool.tile([128, TOK], BF, tag=f"tabr{j}", name=f"tabr{j}") for j in range(4)]
        tab_f = [misc1_pool.tile([128, TOK], BF, tag=f"tabf{j}", name=f"tabf{j}") for j in range(4)]
        for ci in range(8):
            nc.scalar.dma_start(
                out=xt_sb[:, ci, :],
                in_=xT.ap()[ci * 128:(ci + 1) * 128, :],
            )
        nc.scalar.dma_start(out=bias_sb[:], in_=bqkv.ap())
        nc.scalar.dma_start(
            out=wsum_sb[:], in_=wsum.ap().rearrange("(c p) w -> p c w", p=128)
        )
        nc.scalar.dma_start(out=swp_sb[:], in_=swp.ap())
        for j in range(4):
            nc.scalar.dma_start(out=tab_r[j][:], in_=tabs.ap()[j])

        def qkv_chunk_psum(co):
            """psum[128ch, TOK] = sum_ci W[ci, co].T @ xT[ci] + b[co] (x) ones."""
            ps = qkvp.tile([128, TOK], F32, tag="qkvps", name="qkvps")
            wt = wq_pool.tile([128, 8, 128], BF, tag="wt", name="wt")
            nc.sync.dma_start(
                out=wt[:],
                in_=wqkv.ap()[:, co * 128:(co + 1) * 128].rearrange(
                    "(c p) m -> p c m", p=128
                ),
            )
            for ci in range(8):
                nc.tensor.matmul(
                    ps[:], wt[:, ci, :], xt_sb[:, ci, :],
                    start=(ci == 0), stop=(zero_bias and ci == 7),
                )
            if zero_bias:
                pass
            else:
                nc.tensor.matmul(
                    ps[:], bias_sb[:, co * 128:(co + 1) * 128], ones_t[:],
                    start=False, stop=True,
                )
            return ps

        def norm_rope_tensor(which, dst_tiles):
            """which: 0 -> q, 1 -> k. Writes 8 roped bf16 chunks to dst_tiles."""
            co0 = which * 8
            qt_list = []
            ssq = ssqp.tile([1, TOK], F32, tag="ssq", name="ssq")
            for c in range(8):
                ps = qkv_chunk_psum(co0 + c)
                qt = qtsb_pool.tile([128, TOK], BF, tag="qt", name="qt")
                nc.vector.tensor_copy(qt[:], ps[:])
                sqt = sq_pool.tile([128, TOK], BF, tag="sqt", name="sqt")
                nc.vector.tensor_mul(sqt[:], qt[:], qt[:])
                nc.tensor.matmul(
                    ssq[:], wsum_sb[:, c, which:which + 1], sqt[:],
                    start=(c == 0), stop=(c == 7),
                )
                qt_list.append(qt)
            # rstd = exp(-0.5 * ln(var + eps))
            lnv = rstd_pool.tile([1, TOK], F32, tag="lnv", name="lnv")
            nc.scalar.activation(lnv[:], ssq[:], AF.Ln, bias=eps_sb[:])
            rstd = rstd_pool.tile([1, TOK], BF, tag="rstd", name="rstd")
            nc.scalar.activation(rstd[:], lnv[:], AF.Exp, scale=-0.5)
            bc = bcp.tile([128, TOK], F32, tag="bc", name="bc")
            nc.tensor.matmul(bc[:], ones_c[:], rstd[:], start=True, stop=True)
            for j in range(2):
                nc.vector.tensor_mul(
                    tab_f[2 * which + j][:], tab_r[2 * which + j][:], bc[:]
                )
            # rope: dst = qt * C' + (P @ qt) * S'
            for c in range(8):
                sw = swpp.tile([128, TOK], F32, tag="sw", name="sw")
                nc.tensor.matmul(
                    sw[:], swp_sb[:], qt_list[c][:], start=True, stop=True
                )
                m1 = scr1_pool.tile([128, TOK], BF, tag="m1", name="m1")
                nc.vector.tensor_mul(m1[:], qt_list[c][:], tab_f[2 * which][:])
                m2 = scr1_pool.tile([128, TOK], BF, tag="m2", name="m2")
                nc.vector.tensor_mul(m2[:], sw[:], tab_f[2 * which + 1][:])
                nc.vector.tensor_add(dst_tiles[c][:], m1[:], m2[:])
            return qt_list

        vw = vw_pool.tile([128, 8, 2 * TOK], BF, tag="vw", name="vw")
        for ci in range(8):
            nc.sync.dma_start(
                out=vw[:, ci, :],
                in_=wqkv.ap()[ci * 128:(ci + 1) * 128, 2 * DIM:],
            )
        for t4 in range(4):
            for nh in range(2):
                ps = qkvp.tile([128, TOK], F32, tag="qkvps", name="qkvps")
                for ci in range(8):
                    nc.tensor.matmul(
                        ps[:],
                        xt_sb[:, ci, t4 * 128:(t4 + 1) * 128],
                        vw[:, ci, nh * TOK:(nh + 1) * TOK],
                        start=(ci == 0), stop=(zero_bias and ci == 7),
                    )
                if not zero_bias:
                    nc.tensor.matmul(
                        ps[:], ones_c[:],
                        bias_sb[:, 2 * DIM + nh * TOK: 2 * DIM + (nh + 1) * TOK],
                        start=False, stop=True,
                    )
                vl = kvloc_pool.tile([128, TOK], BF, tag=f"vloc{t4}_{nh}", name=f"vloc{t4}_{nh}")
                nc.vector.tensor_copy(vl[:], ps[:])
                dstap = bass.AP(
                    tensor=agin.ap().tensor,
                    offset=1024 * TOK + t4 * 128 * 1024 + nh * TOK,
                    ap=[[1024, 128], [1, TOK]],
                )
                nc.gpsimd.dma_start(out=dstap, in_=vl[:])

        # ---- k first (gates scores), AG-k; v overlaps AG-k; then AG-v, q.
        khat = [kvloc_pool.tile([128, TOK], BF, tag=f"khat{c}", name=f"khat{c}") for c in range(8)]
        norm_rope_tensor(1, khat)
        for c in range(8):
            dstap = bass.AP(
                tensor=agin.ap().tensor,
                offset=c * 128 * TOK,
                ap=[[TOK, 128], [1, TOK]],
            )
            nc.gpsimd.dma_start(out=dstap, in_=khat[c][:])

        if single:
            for r in range(4):
                nc.gpsimd.dma_start(
                    out=bass.AP(tensor=agout.ap().tensor,
                                offset=r * 2048 * TOK, ap=[[1, 2048 * TOK]]),
                    in_=bass.AP(tensor=agin.ap().tensor, offset=0,
                                ap=[[1, 2048 * TOK]]),
                )
        else:
            nc.gpsimd.collective_compute(
                "AllGather",
                mybir.AluOpType.bypass,
                replica_groups=RG,
                ins=[agin.ap().opt()],
                outs=[agout.ap().opt()],
            )

        qt_dbg = norm_rope_tensor(0, qhat)

        def dump8(tiles, cols=None):
            dmp = ExitStack()
            dp = dmp.enter_context(tc.tile_pool(name="dump", bufs=2))
            for c, t in enumerate(tiles):
                f = dp.tile([t.shape[0], TOK], F32, tag="dmp", name="dmp")
                srcap = t[:, cols] if cols is not None else t[:]
                nc.vector.tensor_copy(f[:], srcap)
                nc.gpsimd.dma_start(
                    out=dbgt.ap()[c * 128:c * 128 + t.shape[0], :], in_=f[:]
                )
            dmp.close()

        if dbg == "qt":
            dump8(qt_dbg)
        if dbg == "qhat":
            dump8(qhat)
        if dbg == "khat":
            dump8(khat)

        # ---- post-AG loads
        RSZ = 2048 * TOK
        for c in range(8):
            for r in range(4):
                srcap = bass.AP(
                    tensor=agout.ap().tensor,
                    offset=r * RSZ + c * 128 * TOK,
                    ap=[[TOK, 128], [1, TOK]],
                )
                nc.sync.dma_start(
                    out=ktf[c][:, r * TOK:(r + 1) * TOK], in_=srcap
                )
        for r in range(4):
            for t4 in range(4):
                vc = r * 4 + t4
                srcap = bass.AP(
                    tensor=agout.ap().tensor,
                    offset=r * RSZ + 1024 * TOK + t4 * 128 * 1024,
                    ap=[[1024, 128], [64, 16], [1, 64]],
                )
                dst = vaug[vc][:].rearrange("p (h c) -> p h c", c=65)
                nc.sync.dma_start(out=dst[:, :, 0:64], in_=srcap)
                nc.vector.memset(dst[:, :, 64:65], 1.0)

        if dbg == "ktf":
            dump8(ktf, cols=slice(0, TOK))
        if dbg == "vaug":
            dump8(vaug[:8], cols=slice(0, TOK))

        p1.close()

        # ---------------- phase 2: attention ----------------
        p2 = ExitStack()
        wo_res_pool = CTX.enter_context(tc.tile_pool(name="wores", bufs=1))
        wo_pool = CTX.enter_context(tc.tile_pool(name="wo", bufs=3))
        # PSUM: scp 2x2 banks + avp 4x1 = 8
        scp = CTX.enter_context(tc.tile_pool(name="scp", bufs=2, space="PSUM"))
        avp = p2.enter_context(tc.tile_pool(name="avp", bufs=4, space="PSUM"))
        expt_pool = p2.enter_context(tc.tile_pool(name="expt", bufs=20))
        nrm_pool = p2.enter_context(tc.tile_pool(name="nrm", bufs=2))
        ascr_pool = p2.enter_context(tc.tile_pool(name="ascr", bufs=8))

        expt = {}
        av_tiles = {}
        NHG2 = 8   # groups of 2 heads

        def emit_av(hg, kc):
            for hh in range(2):
                h = hg * 2 + hh
                if kc == 0:
                    av_tiles[(hg, hh)] = avp.tile(
                        [65, TOK], F32, tag="av", name="av")
                nc.tensor.matmul(
                    av_tiles[(hg, hh)][:],
                    vaug[kc][:, h * 65:(h + 1) * 65],
                    expt[(hg, kc)][:, hh * TOK:(hh + 1) * TOK],
                    start=(kc == 0), stop=(kc == KC - 1),
                )

        def emit_normalize(hg):
            # sumexp rows live at PSUM partition 64; stage at partition 64,
            # reshape via DRAM to [128,16] for a fast DVE reciprocal, then
            # broadcast-load back.
            se = nrm_pool.tile([65, 2 * TOK], F32, tag="se", name="se")
            for hh in range(2):
                nc.vector.tensor_copy(
                    se[64:65, hh * TOK:(hh + 1) * TOK],
                    av_tiles[(hg, hh)][64:65, :],
                )
            nc.gpsimd.dma_start(
                out=bass.AP(tensor=sescr2.ap().tensor, offset=0,
                            ap=[[1, 2 * TOK]]),
                in_=se[64:65, :])
            sew = nrm_pool.tile([128, 8], F32, tag="sew", name="sew")
            nc.scalar.dma_start(
                out=sew[:],
                in_=bass.AP(tensor=sescr2.ap().tensor, offset=0,
                            ap=[[8, 128], [1, 8]]),
            )
            rw = nrm_pool.tile([128, 8], F32, tag="rw", name="rw")
            nc.vector.reciprocal(out=rw[:], in_=sew[:])
            nc.gpsimd.dma_start(
                out=bass.AP(tensor=sescr.ap().tensor, offset=0,
                            ap=[[8, 128], [1, 8]]),
                in_=rw[:],
            )
            rbc = nrm_pool.tile([64, 2 * TOK], BF, tag="rbc", name="rbc")
            bcast_src = bass.AP(
                tensor=sescr.ap().tensor,
                offset=0,
                ap=[[0, 64], [1, 2 * TOK]],
            )
            nc.gpsimd.dma_start(out=rbc[:], in_=bcast_src)  # casting DMA
            for hh in range(2):
                h = hg * 2 + hh
                a = ascr_pool.tile([64, TOK], BF, tag="ascr", name="ascr")
                nc.vector.tensor_mul(
                    a[:], av_tiles[(hg, hh)][0:64, :],
                    rbc[:, hh * TOK:(hh + 1) * TOK],
                )
                nc.gpsimd.dma_start(
                    out=attnT[h // 2][(h % 2) * 64:(h % 2) * 64 + 64, :],
                    in_=a[:],
                )

        outps_tiles = {}
        for hg in range(NHG2 + 1):
            for kc in range(KC):
                if hg >= 1:
                    emit_av(hg - 1, kc)
                if hg < NHG2:
                    sc = scp.tile([128, 2 * TOK], F32, tag="sc", name="sc")
                    for hh in range(2):
                        h = hg * 2 + hh
                        nc.tensor.matmul(
                            sc[:, hh * TOK:(hh + 1) * TOK],
                            ktf[h // 2][(h % 2) * 64:(h % 2) * 64 + 64,
                                        kc * 128:(kc + 1) * 128],
                            qhat[h // 2][(h % 2) * 64:(h % 2) * 64 + 64, :],
                            start=True, stop=True,
                        )
                    e = expt_pool.tile([128, 2 * TOK], BF, tag="expt",
                                       name="expt")
                    nc.scalar.activation(e[:], sc[:], AF.Exp)
                    expt[(hg, kc)] = e
            if hg == 5:
                # prefetch the output-projection weights during attention
                wo_res = wo_res_pool.tile([128, 8, 1024], BF, tag="wores",
                                          name="wores")
                for ci in range(8):
                    nc.sync.dma_start(
                        out=wo_res[:, ci, :],
                        in_=wout.ap()[ci * 128:(ci + 1) * 128, :],
                    )
            if hg >= 1:
                emit_normalize(hg - 1)
            if dbg == "expt0" and hg == 0:
                dump8([expt[(0, kc)] for kc in range(8)], cols=slice(0, TOK))
            if dbg == "av0" and hg == 1:
                dump8([av_tiles[(0, hh)] for hh in range(2)])
        p2.close()

        # ---------------- phase 3: output projection ----------------
        p3 = ExitStack()
        for co in range(8):
            ps = scp.tile([128, TOK], F32, tag="sc", name="outps")
            for ci in range(8):
                nc.tensor.matmul(
                    ps[:], wo_res[:, ci, co * 128:(co + 1) * 128],
                    attnT[ci][:],
                    start=(ci == 0), stop=(ci == 7),
                )
            osb = wo_pool.tile([128, TOK], F32, tag="osb", name="osb")
            nc.vector.tensor_copy(osb[:], ps[:])
            nc.gpsimd.dma_start(out=out.ap()[co * 128:(co + 1) * 128, :], in_=osb[:])
        p3.close()

    nc.compile()
    return nc


def _host_prep(inputs):
    import ml_dtypes

    bf16 = ml_dtypes.bfloat16
    x = np.asarray(inputs["x"], np.float32)
    Wqkv = np.asarray(inputs["Wqkv"], np.float32)
    bqkv = np.asarray(inputs["bqkv"], np.float32)
    qs = np.asarray(inputs["q_scale"], np.float32)
    ks = np.asarray(inputs["k_scale"], np.float32)
    Wout = np.asarray(inputs["Wout"], np.float32)

    p64 = np.concatenate([np.arange(0, 64, 2), np.arange(1, 64, 2)])
    perm = np.concatenate([64 * h + p64 for h in range(H)])

    qsp, ksp = qs[perm], ks[perm]
    Wq = Wqkv[:, :DIM][:, perm] * qsp[None, :]
    Wk = Wqkv[:, DIM:2 * DIM][:, perm] * ksp[None, :]
    Wv = Wqkv[:, 2 * DIM:]
    W = np.concatenate([Wq, Wk, Wv], 1).astype(bf16)
    bq = bqkv[:DIM][perm] * qsp
    bk = bqkv[DIM:2 * DIM][perm] * ksp
    bias = np.concatenate([bq, bk, bqkv[2 * DIM:]])[None, :].astype(bf16)
    wsum = np.stack(
        [1.0 / (DIM * qsp ** 2), 1.0 / (DIM * ksp ** 2)], 1
    ).astype(bf16)

    sw = np.arange(128)
    swap = np.where(sw % 64 < 32, sw + 32, sw - 32)
    P = np.zeros((128, 128), np.float32)
    P[swap, np.arange(128)] = 1.0  # (P.T @ x)[m] = x[swap[m]]
    P = P.astype(bf16)

    inv_freq = 1.0 / (BASE ** (np.arange(0, HD, 2).astype(np.float32) / HD))
    pos = np.maximum(np.arange(N) - 1, 0).astype(np.float32)
    ang = pos[:, None] * inv_freq[None, :]
    cosT, sinT = np.cos(ang).T, np.sin(ang).T           # (32, N)
    C128 = np.tile(cosT, (4, 1))                         # (128, N)
    S128 = np.concatenate([-sinT, sinT, -sinT, sinT], 0)

    in_maps = []
    for core in range(NCORE):
        b, sh = core // 4, core % 4
        t0 = sh * TOK
        xTs = np.ascontiguousarray(x[b, t0:t0 + TOK, :].T).astype(bf16)
        tabs = np.stack([
            C128[:, t0:t0 + TOK] * 0.125,
            S128[:, t0:t0 + TOK] * 0.125,
            C128[:, t0:t0 + TOK],
            S128[:, t0:t0 + TOK],
        ]).astype(bf16)
        in_maps.append({
            "xT": xTs,
            "wqkv": W,
            "bqkv": bias,
            "wsum": wsum,
            "swp": P,
            "tabs": np.ascontiguousarray(tabs),
            "wout": Wout.astype(bf16),
        })
    return in_maps


LAST_EXEC_NS = None


def kernel(**inputs):
    global LAST_EXEC_NS
    import os
    from concourse.bass_utils import run_bass_kernel_spmd

    dbg = os.environ.get("KERNEL_DBG") or None
    zb = bool(np.all(np.asarray(inputs["bqkv"]) == 0))
    key = f"nc{dbg}{zb}"
    if key not in _CACHE:
        _CACHE[key] = _build_nc(dbg, zero_bias=zb)
    nc = _CACHE[key]

    in_maps = _host_prep(inputs)
    trace = bool(int(os.environ.get("KERNEL_TRACE", "0")))
    tmpdir = None
    if trace:
        import tempfile
        import concourse.bass_utils as _bu
        _bu.upload_artifacts = lambda d: d  # keep artifacts local
        tmpdir = tempfile.mkdtemp(prefix="ktrace_")
        print("TRACE DIR:", tmpdir)
    res = run_bass_kernel_spmd(
        nc, in_maps, core_ids=list(range(NCORE)), trace=trace, tmpdir=tmpdir
    )
    LAST_EXEC_NS = res.exec_time_ns
    bout = np.asarray(inputs["bout"], np.float32)
    out = np.empty((B, N, DIN), np.float32)
    for core in range(NCORE):
        b, sh = core // 4, core % 4
        t0 = sh * TOK
        out[b, t0:t0 + TOK, :] = res.results[core]["out"].T
    out += bout[None, None, :]
    return out


def kernel_raw(inputs):
    """Debug helper: run and return the per-core raw [1024, 512] outputs."""
    global LAST_EXEC_NS
    import os
    from concourse.bass_utils import run_bass_kernel_spmd

    dbg = os.environ.get("KERNEL_DBG") or None
    zb = bool(np.all(np.asarray(inputs["bqkv"]) == 0))
    key = f"nc{dbg}{zb}"
    if key not in _CACHE:
        _CACHE[key] = _build_nc(dbg, zero_bias=zb)
    nc = _CACHE[key]
    in_maps = _host_prep(inputs)
    res = run_bass_kernel_spmd(nc, in_maps, core_ids=list(range(NCORE)))
    LAST_EXEC_NS = res.exec_time_ns
    key = "dbg" if dbg else "out"
    return [r[key] for r in res.results]



# revision 5
# speedup vs baseline: 1.0080x; 1.0080x over previous
"""Trainium2 8-core Bass kernel for nn_Attention_7112465842253.

Token-sharded attention: 512 tokens/core (cores 0-3 = batch 0, 4-7 = batch 1).
fp16 throughout (same PE cost as bf16, ~8x lower quantization error).
Per core: QKV projection (q/k in transposed [chan, tok] layout, v in natural
[tok, chan] pre-augmented with a ones column for the softmax denominator),
RMSNorm via weighted-sumsq matmul with the rstd folded into the RoPE tables,
RoPE as x*C + (P@x)*S with a PE partition-swap matmul, split AllGather of K
then V inside each 4-core batch group, non-causal attention in scoresT layout
(k-tokens on partitions; the 1/8 score scale folded into the exp, which is
computed on BOTH the Activation engine (Exp activation) and the Pool engine
(tensor_tensor pow with base e^0.125) to break the single-engine exp wall),
softmax denominator from the ones column of V via the attn@V matmul,
normalization via DVE reciprocal + gpsimd partition_broadcast (no DRAM
round-trips), then the output projection. Host does layout prep/reassembly.
"""

import numpy as np

B, N, DIN, DIM, H, HD = 2, 2048, 1024, 1024, 16, 64
NCORE = 8
TOK = 512
EPS = 1e-6
BASE = 10000.0
KC = 16              # 128-token k chunks
KREG = 8 * 128 * TOK          # elements in k AG region per core
VREG = TOK * 16 * 65          # elements in v AG region per core (augmented)

_CACHE = {}


def _build_nc(dbg=None, single=False, zero_bias=False):
    import concourse.bass as bass
    import concourse.tile as tile
    from concourse import bacc, mybir
    from contextlib import ExitStack

    F16 = mybir.dt.float16
    F32 = mybir.dt.float32
    AF = mybir.ActivationFunctionType
    ALU = mybir.AluOpType

    nc = bacc.Bacc(
        "TRN2", target_bir_lowering=False, debug=False,
        num_devices=(1 if single else NCORE),
    )

    # ---------------- DRAM parameters ----------------
    xtd = nc.dram_tensor("xt", [128, 8, TOK], F16, kind="ExternalInput")
    wqkd = nc.dram_tensor("wqk", [16, 128, 8, 128], F16, kind="ExternalInput")
    wvd = nc.dram_tensor("wv", [128, 8, DIM], F16, kind="ExternalInput")
    wod = nc.dram_tensor("wo", [128, 8, DIN], F16, kind="ExternalInput")
    bqkv = nc.dram_tensor("bqkv", [1, 3 * DIM], F16, kind="ExternalInput")
    wsum = nc.dram_tensor("wsum", [128, 8, 2], F16, kind="ExternalInput")
    swp = nc.dram_tensor("swp", [128, 128], F16, kind="ExternalInput")
    tabs = nc.dram_tensor("tabs", [2, 128, TOK], F16, kind="ExternalInput")
    out = nc.dram_tensor("out", [DIN, TOK], F32, kind="ExternalOutput")
    dbgt = (nc.dram_tensor("dbg", [DIN, 2 * TOK], F32, kind="ExternalOutput")
            if dbg else None)

    # internal DRAM for the AllGathers (fp16)
    agink = nc.dram_tensor("agink", [KREG], F16)
    aginv = nc.dram_tensor("aginv", [VREG], F16)
    agoutk = nc.dram_tensor("agoutk", [4 * KREG], F16)
    agoutv = nc.dram_tensor("agoutv", [4 * VREG], F16)

    RG = [[0, 1, 2, 3], [4, 5, 6, 7]]

    with tile.TileContext(nc) as tc, ExitStack() as CTX:
        # ---------------- persistent SBUF ----------------
        pp = CTX.enter_context(tc.tile_pool(name="persist", bufs=1))
        qhat = [pp.tile([128, TOK], F16, tag=f"qhat{c}", name=f"qhat{c}")
                for c in range(8)]
        ktf = [pp.tile([128, N], F16, tag=f"ktf{c}", name=f"ktf{c}")
               for c in range(8)]
        vaug = [pp.tile([128, 16, 65], F16, tag=f"va{c}", name=f"va{c}")
                for c in range(KC)]
        attnT = [pp.tile([128, TOK], F16, tag=f"at{c}", name=f"at{c}")
                 for c in range(8)]

        bias_sb = pp.tile([1, 3 * DIM], F16, tag="bias_sb", name="bias_sb")
        wsum_sb = pp.tile([128, 8, 2], F16, tag="wsum_sb", name="wsum_sb")
        swp_sb = pp.tile([128, 128], F16, tag="swp_sb", name="swp_sb")
        tabr = [pp.tile([128, TOK], F16, tag=f"tabr{j}", name=f"tabr{j}")
                for j in range(2)]
        tabf = [pp.tile([128, TOK], F16, tag=f"tabf{j}", name=f"tabf{j}")
                for j in range(4)]
        ones_t = pp.tile([1, TOK], F16, tag="ones_t", name="ones_t")
        ones_c = pp.tile([1, 128], F16, tag="ones_c", name="ones_c")
        eps_sb = pp.tile([1, 1], F32, tag="eps_sb", name="eps_sb")
        ebase = pp.tile([128, 2 * TOK], F16, tag="ebase", name="ebase")

        nc.vector.memset(ones_t[:], 1.0)
        nc.vector.memset(ones_c[:], 1.0)
        nc.vector.memset(eps_sb[:], EPS)
        nc.vector.memset(ebase[:], float(np.exp(0.125)))

        # ---------------- phase 1: qkv + norm + rope + AG ----------------
        p1 = ExitStack()
        wq_pool = p1.enter_context(tc.tile_pool(name="wq", bufs=4))
        vw_pool = p1.enter_context(tc.tile_pool(name="vw", bufs=1))
        misc1_pool = p1.enter_context(tc.tile_pool(name="misc1", bufs=1))
        qtsb_pool = p1.enter_context(tc.tile_pool(name="qtsb", bufs=10))
        sq_pool = p1.enter_context(tc.tile_pool(name="sq", bufs=2))
        scr1_pool = p1.enter_context(tc.tile_pool(name="scr1", bufs=4))
        kvloc_pool = p1.enter_context(tc.tile_pool(name="kvloc", bufs=1))
        rstd_pool = p1.enter_context(tc.tile_pool(name="rstd", bufs=4))
        # PSUM budget (8 banks): qkvp 3 + swpp 3 + ssqp 1 + bcp 1
        qkvp = p1.enter_context(tc.tile_pool(name="qkvp", bufs=3, space="PSUM"))
        swpp = p1.enter_context(tc.tile_pool(name="swpp", bufs=3, space="PSUM"))
        ssqp = p1.enter_context(tc.tile_pool(name="ssqp", bufs=1, space="PSUM"))
        bcp = p1.enter_context(tc.tile_pool(name="bcp", bufs=1, space="PSUM"))

        xt_sb = misc1_pool.tile([128, 8, TOK], F16, tag="xt_sb", name="xt_sb")
        nc.sync.dma_start(out=xt_sb[:], in_=xtd.ap())
        nc.scalar.dma_start(out=bias_sb[:], in_=bqkv.ap())
        nc.scalar.dma_start(out=wsum_sb[:], in_=wsum.ap())
        nc.scalar.dma_start(out=swp_sb[:], in_=swp.ap())
        for j in range(2):
            nc.scalar.dma_start(out=tabr[j][:], in_=tabs.ap()[j])

        def qk_chunk(co):
            """psum[128ch, TOK] = W[:, co].T @ xT + b[co] (x) ones."""
            ps = qkvp.tile([128, TOK], F32, tag="qkvps", name="qkvps")
            wt = wq_pool.tile([128, 8, 128], F16, tag="wt", name="wt")
            nc.sync.dma_start(out=wt[:], in_=wqkd.ap()[co])
            for ci in range(8):
                nc.tensor.matmul(
                    ps[:], wt[:, ci, :], xt_sb[:, ci, :],
                    start=(ci == 0), stop=(zero_bias and ci == 7),
                )
            if not zero_bias:
                nc.tensor.matmul(
                    ps[:], bias_sb[:, co * 128:(co + 1) * 128], ones_t[:],
                    start=False, stop=True,
                )
            return ps

        def norm_side(which, dst_tiles):
            """which: 0 -> q, 1 -> k. Writes 8 roped fp16 chunks."""
            co0 = which * 8
            ssq = ssqp.tile([1, TOK], F32, tag="ssq", name="ssq")
            qts = []
            for c in range(8):
                ps = qk_chunk(co0 + c)
                qt = qtsb_pool.tile([128, TOK], F16, tag="qt", name="qt")
                nc.scalar.copy(qt[:], ps[:])
                sqt = sq_pool.tile([128, TOK], F16, tag="sqt", name="sqt")
                nc.vector.tensor_mul(sqt[:], qt[:], qt[:])
                nc.tensor.matmul(
                    ssq[:], wsum_sb[:, c, which:which + 1], sqt[:],
                    start=(c == 0), stop=(c == 7),
                )
                qts.append(qt)
            # rstd = exp(-0.5 * ln(var + eps))
            lnv = rstd_pool.tile([1, TOK], F32, tag="lnv", name="lnv")
            nc.scalar.activation(lnv[:], ssq[:], AF.Ln, bias=eps_sb[:])
            rstd = rstd_pool.tile([1, TOK], F16, tag="rstd", name="rstd")
            nc.scalar.activation(rstd[:], lnv[:], AF.Exp, scale=-0.5)
            bc = bcp.tile([128, TOK], F32, tag="bc", name="bc")
            nc.tensor.matmul(bc[:], ones_c[:], rstd[:], start=True, stop=True)
            cf, sf = tabf[2 * which], tabf[2 * which + 1]
            nc.vector.tensor_mul(cf[:], tabr[0][:], bc[:])
            nc.vector.tensor_mul(sf[:], tabr[1][:], bc[:])
            # rope: dst = qt * C' + (P @ qt) * S'
            for c in range(8):
                sw = swpp.tile([128, TOK], F32, tag="sw", name="sw")
                nc.tensor.matmul(sw[:], swp_sb[:], qts[c][:],
                                 start=True, stop=True)
                m1 = scr1_pool.tile([128, TOK], F16, tag="m1", name="m1")
                nc.vector.tensor_mul(m1[:], qts[c][:], cf[:])
                m2 = scr1_pool.tile([128, TOK], F16, tag="m2", name="m2")
                nc.vector.tensor_mul(m2[:], sw[:], sf[:])
                nc.vector.tensor_add(dst_tiles[c][:], m1[:], m2[:])
            return qts

        # ---- k first (gates AG-k); v next; q overlaps the AllGathers
        khat = [kvloc_pool.tile([128, TOK], F16, tag=f"khat{c}",
                                name=f"khat{c}") for c in range(8)]
        norm_side(1, khat)
        for c in range(8):
            dstap = bass.AP(
                tensor=agink.ap().tensor,
                offset=c * 128 * TOK,
                ap=[[TOK, 128], [1, TOK]],
            )
            nc.gpsimd.dma_start(out=dstap, in_=khat[c][:])
        if single:
            for r in range(4):
                nc.gpsimd.dma_start(
                    out=bass.AP(tensor=agoutk.ap().tensor,
                                offset=r * KREG, ap=[[1, KREG]]),
                    in_=bass.AP(tensor=agink.ap().tensor, offset=0,
                                ap=[[1, KREG]]),
                )
        else:
            nc.gpsimd.collective_compute(
                "AllGather", mybir.AluOpType.bypass, replica_groups=RG,
                ins=[agink.ap().opt()], outs=[agoutk.ap().opt()],
            )

        # ---- v: natural [tok, chan] with a ones column per head
        vw = vw_pool.tile([128, 8, 2 * TOK], F16, tag="vw", name="vw")
        nc.sync.dma_start(out=vw[:], in_=wvd.ap())
        for t4 in range(4):
            vl = kvloc_pool.tile([128, 16, 65], F16, tag=f"vl{t4}",
                                 name=f"vl{t4}")
            for nh in range(2):
                ps = qkvp.tile([128, TOK], F32, tag="qkvps", name="qkvps")
                for ci in range(8):
                    nc.tensor.matmul(
                        ps[:],
                        xt_sb[:, ci, t4 * 128:(t4 + 1) * 128],
                        vw[:, ci, nh * TOK:(nh + 1) * TOK],
                        start=(ci == 0), stop=(zero_bias and ci == 7),
                    )
                if not zero_bias:
                    nc.tensor.matmul(
                        ps[:], ones_c[:],
                        bias_sb[:, 2 * DIM + nh * TOK:
                                2 * DIM + (nh + 1) * TOK],
                        start=False, stop=True,
                    )
                nc.scalar.copy(
                    vl[:, 8 * nh:8 * nh + 8, 0:64],
                    ps[:].rearrange("p (h d) -> p h d", d=64),
                )
            nc.vector.memset(vl[:, :, 64:65], 1.0)
            dstap = bass.AP(
                tensor=aginv.ap().tensor,
                offset=t4 * 128 * 16 * 65,
                ap=[[16 * 65, 128], [1, 16 * 65]],
            )
            nc.gpsimd.dma_start(out=dstap, in_=vl[:])
        if single:
            for r in range(4):
                nc.gpsimd.dma_start(
                    out=bass.AP(tensor=agoutv.ap().tensor,
                                offset=r * VREG, ap=[[1, VREG]]),
                    in_=bass.AP(tensor=aginv.ap().tensor, offset=0,
                                ap=[[1, VREG]]),
                )
        else:
            nc.gpsimd.collective_compute(
                "AllGather", mybir.AluOpType.bypass, replica_groups=RG,
                ins=[aginv.ap().opt()], outs=[agoutv.ap().opt()],
            )

        qt_dbg = norm_side(0, qhat)

        def dump(tiles, w=None):
            dmp = ExitStack()
            dp = dmp.enter_context(tc.tile_pool(name="dump", bufs=2))
            for c, t in enumerate(tiles):
                ww = w or min(t.shape[-1], 2 * TOK)
                f = dp.tile([t.shape[0], ww], F32, tag="dmp", name="dmp")
                nc.vector.tensor_copy(f[:], t[:, :ww])
                nc.gpsimd.dma_start(
                    out=dbgt.ap()[c * t.shape[0]:(c + 1) * t.shape[0], :ww],
                    in_=f[:])
            dmp.close()

        # ---- post-AG loads: ktf in head-pair order so scores start early
        for c in range(8):
            for r in range(4):
                srcap = bass.AP(
                    tensor=agoutk.ap().tensor,
                    offset=r * KREG + c * 128 * TOK,
                    ap=[[TOK, 128], [1, TOK]],
                )
                nc.sync.dma_start(out=ktf[c][:, r * TOK:(r + 1) * TOK],
                                  in_=srcap)
        for kc in range(KC):
            r, t4 = kc // 4, kc % 4
            srcap = bass.AP(
                tensor=agoutv.ap().tensor,
                offset=r * VREG + t4 * 128 * 16 * 65,
                ap=[[16 * 65, 128], [1, 16 * 65]],
            )
            nc.sync.dma_start(
                out=vaug[kc][:].rearrange("p h d -> p (h d)"), in_=srcap)

        if dbg == "qhat":
            dump(qhat)
        if dbg == "khat":
            dump(khat)
        if dbg == "qt":
            dump(qt_dbg)

        p1.close()

        # ---------------- phase 2: attention ----------------
        p2 = ExitStack()
        wo_res_pool = CTX.enter_context(tc.tile_pool(name="wores", bufs=1))
        wo_pool = CTX.enter_context(tc.tile_pool(name="wo", bufs=3))
        # PSUM: scp 2x2 banks + avp 4x1 = 8
        scp = CTX.enter_context(tc.tile_pool(name="scp", bufs=2, space="PSUM"))
        avp = p2.enter_context(tc.tile_pool(name="avp", bufs=4, space="PSUM"))
        expt_pool = p2.enter_context(tc.tile_pool(name="expt", bufs=20))
        stg_pool = p2.enter_context(tc.tile_pool(name="stg", bufs=3))
        nrm_pool = p2.enter_context(tc.tile_pool(name="nrm", bufs=2))
        ascr_pool = p2.enter_context(tc.tile_pool(name="ascr", bufs=4))

        expt = {}
        av_tiles = {}

        def emit_scores(hg, kc):
            sc = scp.tile([128, 2 * TOK], F32, tag="sc", name="sc")
            for hh in range(2):
                nc.tensor.matmul(
                    sc[:, hh * TOK:(hh + 1) * TOK],
                    ktf[hg][hh * 64:hh * 64 + 64, kc * 128:(kc + 1) * 128],
                    qhat[hg][hh * 64:hh * 64 + 64, :],
                    start=True, stop=True,
                )
            e = expt_pool.tile([128, 2 * TOK], F16, tag="expt", name="expt")
            if kc % 4 == 3:
                # exp on the Pool engine (which cannot read PSUM): DVE
                # stages the scores to SBUF, then e = (e^0.125) ** score.
                scf = stg_pool.tile([128, 2 * TOK], F16, tag="scf",
                                    name="scf")
                nc.vector.tensor_copy(scf[:], sc[:])
                nc.gpsimd.tensor_tensor(out=e[:], in0=ebase[:], in1=scf[:],
                                        op=ALU.pow)
            else:
                nc.scalar.activation(e[:], sc[:], AF.Exp, scale=0.125)
            expt[(hg, kc)] = e

        def emit_av(hg, kc):
            for hh in range(2):
                h = 2 * hg + hh
                if kc == 0:
                    av_tiles[(hg, hh)] = avp.tile(
                        [65, TOK], F32, tag="av", name="av")
                nc.tensor.matmul(
                    av_tiles[(hg, hh)][:],
                    vaug[kc][:, h, :],
                    expt[(hg, kc)][:, hh * TOK:(hh + 1) * TOK],
                    start=(kc == 0), stop=(kc == KC - 1),
                )

        def emit_normalize(hg):
            # denominators sit on psum partition 64: reciprocal on DVE,
            # partition_broadcast on gpsimd, multiply+evict on DVE.
            rw1 = nrm_pool.tile([1, 2 * TOK], F32, tag="rw1", name="rw1")
            for hh in range(2):
                nc.vector.reciprocal(
                    out=rw1[:, hh * TOK:(hh + 1) * TOK],
                    in_=av_tiles[(hg, hh)][64:65, :])
            rbc = nrm_pool.tile([64, 2 * TOK], F32, tag="rbc", name="rbc")
            nc.gpsimd.partition_broadcast(rbc[:], rw1[:])
            for hh in range(2):
                a8 = ascr_pool.tile([64, TOK], F16, tag="a8", name="a8")
                nc.vector.tensor_mul(
                    a8[:], av_tiles[(hg, hh)][0:64, :],
                    rbc[:, hh * TOK:(hh + 1) * TOK],
                )
                nc.gpsimd.dma_start(
                    out=attnT[hg][64 * hh:64 * hh + 64, :], in_=a8[:])

        for hg in range(9):
            for kc in range(KC):
                if hg >= 1:
                    emit_av(hg - 1, kc)
                if hg < 8:
                    emit_scores(hg, kc)
            if hg == 5:
                wo_res = wo_res_pool.tile([128, 8, DIN], F16, tag="wores",
                                          name="wores")
                nc.sync.dma_start(out=wo_res[:], in_=wod.ap())
            if hg >= 1:
                emit_normalize(hg - 1)
            if dbg == "expt0" and hg == 1:
                dump([expt[(0, kc)] for kc in range(8)])
        p2.close()

        if dbg == "attnT":
            dump(attnT, w=TOK)

        # ---------------- phase 3: output projection ----------------
        for co in range(8):
            ps = scp.tile([128, TOK], F32, tag="sc", name="outps")
            for ci in range(8):
                nc.tensor.matmul(
                    ps[:], wo_res[:, ci, co * 128:(co + 1) * 128],
                    attnT[ci][:],
                    start=(ci == 0), stop=(ci == 7),
                )
            osb = wo_pool.tile([128, TOK], F32, tag="osb", name="osb")
            nc.vector.tensor_copy(osb[:], ps[:])
            nc.gpsimd.dma_start(out=out.ap()[co * 128:(co + 1) * 128, :],
                                in_=osb[:])

    nc.compile()
    return nc


def _host_prep(inputs):
    import ml_dtypes

    f16 = np.float16
    x = np.asarray(inputs["x"], np.float32)
    Wqkv = np.asarray(inputs["Wqkv"], np.float32)
    bqkv = np.asarray(inputs["bqkv"], np.float32)
    qs = np.asarray(inputs["q_scale"], np.float32)
    ks = np.asarray(inputs["k_scale"], np.float32)
    Wout = np.asarray(inputs["Wout"], np.float32)

    p64 = np.concatenate([np.arange(0, 64, 2), np.arange(1, 64, 2)])
    perm = np.concatenate([64 * h + p64 for h in range(H)])

    qsp, ksp = qs[perm], ks[perm]
    Wq = (Wqkv[:, :DIM] * qs[None, :])[:, perm]
    Wk = (Wqkv[:, DIM:2 * DIM] * ks[None, :])[:, perm]
    Wqk = np.concatenate([Wq, Wk], 1)                    # [1024, 2048]
    wqk = np.ascontiguousarray(
        Wqk.reshape(8, 128, 16, 128).transpose(2, 1, 0, 3)).astype(f16)
    wv = np.ascontiguousarray(
        Wqkv[:, 2 * DIM:].reshape(8, 128, DIM).transpose(1, 0, 2)).astype(f16)
    wo = np.ascontiguousarray(
        Wout.reshape(8, 128, DIN).transpose(1, 0, 2)).astype(f16)

    wsum = np.stack([1.0 / (DIM * qsp ** 2), 1.0 / (DIM * ksp ** 2)], 1)
    wsum = np.ascontiguousarray(
        wsum.reshape(8, 128, 2).transpose(1, 0, 2)).astype(f16)

    bq = (bqkv[:DIM] * qs)[perm]
    bk = (bqkv[DIM:2 * DIM] * ks)[perm]
    bias = np.concatenate([bq, bk, bqkv[2 * DIM:]])[None, :].astype(f16)

    sw = np.arange(128)
    swap = np.where(sw % 64 < 32, sw + 32, sw - 32)
    P = np.zeros((128, 128), np.float32)
    P[swap, np.arange(128)] = 1.0  # (P.T @ x)[m] = x[swap[m]]
    P = P.astype(f16)

    inv_freq = 1.0 / (BASE ** (np.arange(0, HD, 2).astype(np.float32) / HD))
    pos = np.maximum(np.arange(N) - 1, 0).astype(np.float32)
    ang = pos[:, None] * inv_freq[None, :]
    cosT, sinT = np.cos(ang).T, np.sin(ang).T           # (32, N)
    C128 = np.tile(cosT, (4, 1))                        # (128, N)
    S128 = np.concatenate([-sinT, sinT, -sinT, sinT], 0)

    in_maps = []
    for core in range(NCORE):
        b, sh = core // 4, core % 4
        t0 = sh * TOK
        xt = np.ascontiguousarray(
            x[b, t0:t0 + TOK, :].T.reshape(8, 128, TOK).transpose(1, 0, 2)
        ).astype(f16)
        tabsc = np.ascontiguousarray(
            np.stack([C128[:, t0:t0 + TOK], S128[:, t0:t0 + TOK]])
        ).astype(f16)
        in_maps.append({
            "xt": xt,
            "wqk": wqk,
            "wv": wv,
            "wo": wo,
            "bqkv": bias,
            "wsum": wsum,
            "swp": P,
            "tabs": tabsc,
        })
    return in_maps


LAST_EXEC_NS = None


def kernel(**inputs):
    global LAST_EXEC_NS
    import os
    from concourse.bass_utils import run_bass_kernel_spmd

    dbg = os.environ.get("KERNEL_DBG") or None
    zb = bool(np.all(np.asarray(inputs["bqkv"]) == 0))
    key = f"nc{dbg}{zb}"
    if key not in _CACHE:
        _CACHE[key] = _build_nc(dbg, zero_bias=zb)
    nc = _CACHE[key]

    in_maps = _host_prep(inputs)
    trace = bool(int(os.environ.get("KERNEL_TRACE", "0")))
    tmpdir = None
    if trace:
        import tempfile
        import concourse.bass_utils as _bu
        _bu.upload_artifacts = lambda d: d  # keep artifacts local
        tmpdir = tempfile.mkdtemp(prefix="ktrace_")
        print("TRACE DIR:", tmpdir)
    res = run_bass_kernel_spmd(
        nc, in_maps, core_ids=list(range(NCORE)), trace=trace, tmpdir=tmpdir
    )
    LAST_EXEC_NS = res.exec_time_ns
    bout = np.asarray(inputs["bout"], np.float32)
    out = np.empty((B, N, DIN), np.float32)
    for core in range(NCORE):
        b, sh = core // 4, core % 4
        t0 = sh * TOK
        out[b, t0:t0 + TOK, :] = res.results[core]["out"].T
    out += bout[None, None, :]
    return out


def kernel_raw(inputs):
    """Debug helper: run and return the per-core raw outputs."""
    global LAST_EXEC_NS
    import os
    from concourse.bass_utils import run_bass_kernel_spmd

    dbg = os.environ.get("KERNEL_DBG") or None
    zb = bool(np.all(np.asarray(inputs["bqkv"]) == 0))
    key = f"nc{dbg}{zb}"
    if key not in _CACHE:
        _CACHE[key] = _build_nc(dbg, zero_bias=zb)
    nc = _CACHE[key]
    in_maps = _host_prep(inputs)
    res = run_bass_kernel_spmd(nc, in_maps, core_ids=list(range(NCORE)))
    LAST_EXEC_NS = res.exec_time_ns
    key = "dbg" if dbg else "out"
    return [r[key] for r in res.results]


# revision 11
# speedup vs baseline: 1.0676x; 1.0591x over previous
"""Trainium2 8-core Bass kernel for nn_Attention_7112465842253.

Token-sharded attention: 512 tokens/core (cores 0-3 = batch 0, 4-7 = batch 1).
fp16 throughout (same PE cost as bf16, ~8x lower quantization error).
Per core: QKV projection (q/k in transposed [chan, tok] layout, v in natural
[tok, chan] pre-augmented with a ones column for the softmax denominator),
RMSNorm via weighted-sumsq matmul with the rstd folded into the RoPE tables,
RoPE as x*C + (P@x)*S with a PE partition-swap matmul, split AllGather of K
then V inside each 4-core batch group, non-causal attention in scoresT layout
(k-tokens on partitions; the 1/8 score scale folded into the exp, which is
computed on BOTH the Activation engine (Exp activation) and the Pool engine
(tensor_tensor pow with base e^0.125) to break the single-engine exp wall),
softmax denominator from the ones column of V via the attn@V matmul,
normalization via DVE reciprocal + gpsimd partition_broadcast (no DRAM
round-trips), then the output projection. Host does layout prep/reassembly.
"""

import numpy as np

B, N, DIN, DIM, H, HD = 2, 2048, 1024, 1024, 16, 64
NCORE = 8
TOK = 512
EPS = 1e-6
BASE = 10000.0
KC = 16              # 128-token k chunks
KREG = 8 * 128 * TOK          # elements in k AG region per core
VREG = TOK * 16 * 65          # elements in v AG region per core (augmented)

_CACHE = {}


def _build_nc(dbg=None, single=False, zero_bias=False):
    import concourse.bass as bass
    import concourse.tile as tile
    from concourse import bacc, mybir
    from contextlib import ExitStack

    F16 = mybir.dt.float16
    F32 = mybir.dt.float32
    AF = mybir.ActivationFunctionType
    ALU = mybir.AluOpType

    nc = bacc.Bacc(
        "TRN2", target_bir_lowering=False, debug=False,
        num_devices=(1 if single else NCORE),
    )

    # ---------------- DRAM parameters ----------------
    xtd = nc.dram_tensor("xt", [128, 8, TOK], F16, kind="ExternalInput")
    wqkd = nc.dram_tensor("wqk", [16, 128, 8, 128], F16, kind="ExternalInput")
    wvd = nc.dram_tensor("wv", [128, 8, DIM], F16, kind="ExternalInput")
    wod = nc.dram_tensor("wo", [128, 8, DIN], F16, kind="ExternalInput")
    bqkv = nc.dram_tensor("bqkv", [1, 3 * DIM], F16, kind="ExternalInput")
    wsum = nc.dram_tensor("wsum", [128, 8, 2], F16, kind="ExternalInput")
    swp = nc.dram_tensor("swp", [128, 128], F16, kind="ExternalInput")
    tabs = nc.dram_tensor("tabs", [2, 128, TOK], F16, kind="ExternalInput")
    out = nc.dram_tensor("out", [DIN, TOK], F32, kind="ExternalOutput")
    dbgt = (nc.dram_tensor("dbg", [DIN, 2 * TOK], F32, kind="ExternalOutput")
            if dbg else None)

    # internal DRAM for the AllGathers (fp16)
    agink = nc.dram_tensor("agink", [KREG], F16)
    aginv = nc.dram_tensor("aginv", [VREG], F16)
    agoutk = nc.dram_tensor("agoutk", [4 * KREG], F16)
    agoutv = nc.dram_tensor("agoutv", [4 * VREG], F16)

    RG = [[0, 1, 2, 3], [4, 5, 6, 7]]

    with tile.TileContext(nc) as tc, ExitStack() as CTX:
        # ---------------- persistent SBUF ----------------
        pp = CTX.enter_context(tc.tile_pool(name="persist", bufs=1))
        qhat = [pp.tile([128, TOK], F16, tag=f"qhat{c}", name=f"qhat{c}")
                for c in range(8)]
        ktf = [pp.tile([128, N], F16, tag=f"ktf{c}", name=f"ktf{c}")
               for c in range(8)]
        vaug = [pp.tile([128, 16, 65], F16, tag=f"va{c}", name=f"va{c}")
                for c in range(KC)]
        attnT = [pp.tile([128, TOK], F16, tag=f"at{c}", name=f"at{c}")
                 for c in range(8)]

        bias_sb = pp.tile([1, 3 * DIM], F16, tag="bias_sb", name="bias_sb")
        wsum_sb = pp.tile([128, 8, 2], F16, tag="wsum_sb", name="wsum_sb")
        swp_sb = pp.tile([128, 128], F16, tag="swp_sb", name="swp_sb")
        tabr = [pp.tile([128, TOK], F16, tag=f"tabr{j}", name=f"tabr{j}")
                for j in range(2)]
        tabf = [pp.tile([128, TOK], F16, tag=f"tabf{j}", name=f"tabf{j}")
                for j in range(4)]
        ones_t = pp.tile([1, TOK], F16, tag="ones_t", name="ones_t")
        ones_c = pp.tile([1, 128], F16, tag="ones_c", name="ones_c")
        ones_p = pp.tile([128, 1], F16, tag="ones_p", name="ones_p")
        eps_sb = pp.tile([1, 1], F32, tag="eps_sb", name="eps_sb")
        ebase = pp.tile([128, 2 * TOK], F16, tag="ebase", name="ebase")

        nc.vector.memset(ones_t[:], 1.0)
        nc.vector.memset(ones_c[:], 1.0)
        nc.vector.memset(ones_p[:], 1.0)
        nc.vector.memset(eps_sb[:], EPS)
        nc.vector.memset(ebase[:], float(np.exp(0.125)))

        # ---------------- phase 1: qkv + norm + rope + AG ----------------
        p1 = ExitStack()
        wq_pool = p1.enter_context(tc.tile_pool(name="wq", bufs=4))
        vw_pool = p1.enter_context(tc.tile_pool(name="vw", bufs=1))
        misc1_pool = p1.enter_context(tc.tile_pool(name="misc1", bufs=1))
        qtsb_pool = p1.enter_context(tc.tile_pool(name="qtsb", bufs=10))
        sq_pool = p1.enter_context(tc.tile_pool(name="sq", bufs=2))
        scr1_pool = p1.enter_context(tc.tile_pool(name="scr1", bufs=4))
        kvloc_pool = p1.enter_context(tc.tile_pool(name="kvloc", bufs=1))
        rstd_pool = p1.enter_context(tc.tile_pool(name="rstd", bufs=4))
        # PSUM budget (8 banks): qkvp 3 + swpp 3 + ssqp 1 + bcp 1
        qkvp = p1.enter_context(tc.tile_pool(name="qkvp", bufs=3, space="PSUM"))
        swpp = p1.enter_context(tc.tile_pool(name="swpp", bufs=3, space="PSUM"))
        ssqp = p1.enter_context(tc.tile_pool(name="ssqp", bufs=1, space="PSUM"))
        bcp = p1.enter_context(tc.tile_pool(name="bcp", bufs=1, space="PSUM"))

        xt_sb = misc1_pool.tile([128, 8, TOK], F16, tag="xt_sb", name="xt_sb")
        nc.sync.dma_start(out=xt_sb[:], in_=xtd.ap())
        nc.scalar.dma_start(out=bias_sb[:], in_=bqkv.ap())
        nc.scalar.dma_start(out=wsum_sb[:], in_=wsum.ap())
        nc.scalar.dma_start(out=swp_sb[:], in_=swp.ap())
        for j in range(2):
            nc.scalar.dma_start(out=tabr[j][:], in_=tabs.ap()[j])

        def qk_chunk(co):
            """psum[128ch, TOK] = W[:, co].T @ xT + b[co] (x) ones."""
            ps = qkvp.tile([128, TOK], F32, tag="qkvps", name="qkvps")
            wt = wq_pool.tile([128, 8, 128], F16, tag="wt", name="wt")
            nc.sync.dma_start(out=wt[:], in_=wqkd.ap()[co])
            for ci in range(8):
                nc.tensor.matmul(
                    ps[:], wt[:, ci, :], xt_sb[:, ci, :],
                    start=(ci == 0), stop=(zero_bias and ci == 7),
                )
            if not zero_bias:
                nc.tensor.matmul(
                    ps[:], bias_sb[:, co * 128:(co + 1) * 128], ones_t[:],
                    start=False, stop=True,
                )
            return ps

        def norm_side(which, dst_tiles):
            """which: 0 -> q, 1 -> k. Writes 8 roped fp16 chunks."""
            co0 = which * 8
            ssq = ssqp.tile([1, TOK], F32, tag="ssq", name="ssq")
            qts = []
            for c in range(8):
                ps = qk_chunk(co0 + c)
                qt = qtsb_pool.tile([128, TOK], F16, tag="qt", name="qt")
                nc.scalar.copy(qt[:], ps[:])
                sqt = sq_pool.tile([128, TOK], F16, tag="sqt", name="sqt")
                nc.vector.tensor_mul(sqt[:], qt[:], qt[:])
                nc.tensor.matmul(
                    ssq[:], wsum_sb[:, c, which:which + 1], sqt[:],
                    start=(c == 0), stop=(c == 7),
                )
                qts.append(qt)
            # rstd = exp(-0.5 * ln(var + eps))
            lnv = rstd_pool.tile([1, TOK], F32, tag="lnv", name="lnv")
            nc.scalar.activation(lnv[:], ssq[:], AF.Ln, bias=eps_sb[:])
            rstd = rstd_pool.tile([1, TOK], F16, tag="rstd", name="rstd")
            nc.scalar.activation(rstd[:], lnv[:], AF.Exp, scale=-0.5)
            bc = bcp.tile([128, TOK], F32, tag="bc", name="bc")
            nc.tensor.matmul(bc[:], ones_c[:], rstd[:], start=True, stop=True)
            cf, sf = tabf[2 * which], tabf[2 * which + 1]
            nc.vector.tensor_mul(cf[:], tabr[0][:], bc[:])
            nc.vector.tensor_mul(sf[:], tabr[1][:], bc[:])
            # rope: dst = qt * C' + (P @ qt) * S'
            for c in range(8):
                sw = swpp.tile([128, TOK], F32, tag="sw", name="sw")
                nc.tensor.matmul(sw[:], swp_sb[:], qts[c][:],
                                 start=True, stop=True)
                m1 = scr1_pool.tile([128, TOK], F16, tag="m1", name="m1")
                nc.vector.tensor_mul(m1[:], qts[c][:], cf[:])
                m2 = scr1_pool.tile([128, TOK], F16, tag="m2", name="m2")
                nc.vector.tensor_mul(m2[:], sw[:], sf[:])
                nc.vector.tensor_add(dst_tiles[c][:], m1[:], m2[:])
            return qts

        # ---- k first (gates AG-k); v next; q overlaps the AllGathers
        khat = [kvloc_pool.tile([128, TOK], F16, tag=f"khat{c}",
                                name=f"khat{c}") for c in range(8)]
        norm_side(1, khat)
        for c in range(8):
            dstap = bass.AP(
                tensor=agink.ap().tensor,
                offset=c * 128 * TOK,
                ap=[[TOK, 128], [1, TOK]],
            )
            nc.gpsimd.dma_start(out=dstap, in_=khat[c][:])
        if single:
            for r in range(4):
                nc.gpsimd.dma_start(
                    out=bass.AP(tensor=agoutk.ap().tensor,
                                offset=r * KREG, ap=[[1, KREG]]),
                    in_=bass.AP(tensor=agink.ap().tensor, offset=0,
                                ap=[[1, KREG]]),
                )
        else:
            nc.gpsimd.collective_compute(
                "AllGather", mybir.AluOpType.bypass, replica_groups=RG,
                ins=[agink.ap().opt()], outs=[agoutk.ap().opt()],
            )

        # ---- v: natural [tok, chan] with a ones column per head
        vw = vw_pool.tile([128, 8, 2 * TOK], F16, tag="vw", name="vw")
        nc.sync.dma_start(out=vw[:], in_=wvd.ap())
        for t4 in range(4):
            vl = kvloc_pool.tile([128, 16, 65], F16, tag=f"vl{t4}",
                                 name=f"vl{t4}")
            for nh in range(2):
                ps = qkvp.tile([128, TOK], F32, tag="qkvps", name="qkvps")
                for ci in range(8):
                    nc.tensor.matmul(
                        ps[:],
                        xt_sb[:, ci, t4 * 128:(t4 + 1) * 128],
                        vw[:, ci, nh * TOK:(nh + 1) * TOK],
                        start=(ci == 0), stop=(zero_bias and ci == 7),
                    )
                if not zero_bias:
                    nc.tensor.matmul(
                        ps[:], ones_c[:],
                        bias_sb[:, 2 * DIM + nh * TOK:
                                2 * DIM + (nh + 1) * TOK],
                        start=False, stop=True,
                    )
                nc.scalar.copy(
                    vl[:, 8 * nh:8 * nh + 8, 0:64],
                    ps[:].rearrange("p (h d) -> p h d", d=64),
                )
            nc.vector.memset(vl[:, :, 64:65], 1.0)
            dstap = bass.AP(
                tensor=aginv.ap().tensor,
                offset=t4 * 128 * 16 * 65,
                ap=[[16 * 65, 128], [1, 16 * 65]],
            )
            nc.gpsimd.dma_start(out=dstap, in_=vl[:])
        if single:
            for r in range(4):
                nc.gpsimd.dma_start(
                    out=bass.AP(tensor=agoutv.ap().tensor,
                                offset=r * VREG, ap=[[1, VREG]]),
                    in_=bass.AP(tensor=aginv.ap().tensor, offset=0,
                                ap=[[1, VREG]]),
                )
        else:
            nc.gpsimd.collective_compute(
                "AllGather", mybir.AluOpType.bypass, replica_groups=RG,
                ins=[aginv.ap().opt()], outs=[agoutv.ap().opt()],
            )

        qt_dbg = norm_side(0, qhat)

        def dump(tiles, w=None):
            dmp = ExitStack()
            dp = dmp.enter_context(tc.tile_pool(name="dump", bufs=2))
            for c, t in enumerate(tiles):
                ww = w or min(t.shape[-1], 2 * TOK)
                f = dp.tile([t.shape[0], ww], F32, tag="dmp", name="dmp")
                nc.vector.tensor_copy(f[:], t[:, :ww])
                nc.gpsimd.dma_start(
                    out=dbgt.ap()[c * t.shape[0]:(c + 1) * t.shape[0], :ww],
                    in_=f[:])
            dmp.close()

        # ---- post-AG loads: ktf in head-pair order so scores start early
        for c in range(8):
            for r in range(4):
                srcap = bass.AP(
                    tensor=agoutk.ap().tensor,
                    offset=r * KREG + c * 128 * TOK,
                    ap=[[TOK, 128], [1, TOK]],
                )
                nc.sync.dma_start(out=ktf[c][:, r * TOK:(r + 1) * TOK],
                                  in_=srcap)
        for kc in range(KC):
            r, t4 = kc // 4, kc % 4
            srcap = bass.AP(
                tensor=agoutv.ap().tensor,
                offset=r * VREG + t4 * 128 * 16 * 65,
                ap=[[16 * 65, 128], [1, 16 * 65]],
            )
            nc.sync.dma_start(
                out=vaug[kc][:].rearrange("p h d -> p (h d)"), in_=srcap)

        if dbg == "qhat":
            dump(qhat)
        if dbg == "khat":
            dump(khat)
        if dbg == "qt":
            dump(qt_dbg)

        p1.close()

        # ---------------- phase 2: attention ----------------
        p2 = ExitStack()
        wo_res_pool = CTX.enter_context(tc.tile_pool(name="wores", bufs=1))
        wo_pool = CTX.enter_context(tc.tile_pool(name="wo", bufs=3))
        # PSUM: scp 2x2 banks + avp 2x1 + denp 2x1 = 8
        scp = CTX.enter_context(tc.tile_pool(name="scp", bufs=2, space="PSUM"))
        avp = p2.enter_context(tc.tile_pool(name="avp", bufs=2, space="PSUM"))
        denp = p2.enter_context(tc.tile_pool(name="denp", bufs=2,
                                             space="PSUM"))
        expt_pool = p2.enter_context(tc.tile_pool(name="expt", bufs=34))
        stg_pool = p2.enter_context(tc.tile_pool(name="stg", bufs=3))
        nrm_pool = p2.enter_context(tc.tile_pool(name="nrm", bufs=8))
        ascr_pool = p2.enter_context(tc.tile_pool(name="ascr", bufs=8))

        expt = {}
        av_tiles = {}
        den_tiles = {}

        def emit_scores(hg, kc):
            sc = scp.tile([128, 2 * TOK], F32, tag="sc", name="sc")
            for hh in range(2):
                nc.tensor.matmul(
                    sc[:, hh * TOK:(hh + 1) * TOK],
                    ktf[hg][hh * 64:hh * 64 + 64, kc * 128:(kc + 1) * 128],
                    qhat[hg][hh * 64:hh * 64 + 64, :],
                    start=True, stop=True,
                )
            e = expt_pool.tile([128, 2 * TOK], F16, tag="expt", name="expt")
            if kc % 4 == 3:
                # exp on the Pool engine (which cannot read PSUM): DVE
                # stages the scores to SBUF, then e = (e^0.125) ** score.
                scf = stg_pool.tile([128, 2 * TOK], F16, tag="scf",
                                    name="scf")
                nc.vector.tensor_copy(scf[:], sc[:])
                nc.gpsimd.tensor_tensor(out=e[:], in0=ebase[:], in1=scf[:],
                                        op=ALU.pow)
            else:
                nc.scalar.activation(e[:], sc[:], AF.Exp, scale=0.125)
            expt[(hg, kc)] = e

        def emit_av_acc(hg, acc):
            # transposed attn@V: probs stationary, v moving. One accumulator
            # slice (hh, qs) at a time, kc-inner — PSUM accumulation groups
            # sharing a bank must not be interleaved. The denominator group
            # (ones moving vector) lives in a different bank so it can
            # interleave with the av group.
            hh, qs = acc // 4, acc % 4
            h = 2 * hg + hh
            if acc == 0:
                av_tiles[hg] = avp.tile([128, 512], F32, tag="av", name="av")
                den_tiles[hg] = denp.tile([128, 8], F32, tag="den",
                                          name="den")
            av, den = av_tiles[hg], den_tiles[hg]
            for kc in range(KC):
                st, sp = (kc == 0), (kc == KC - 1)
                est = expt[(hg, kc)][:, hh * TOK + 128 * qs:
                                     hh * TOK + 128 * (qs + 1)]
                nc.tensor.matmul(
                    av[:, 64 * acc:64 * acc + 64], est,
                    vaug[kc][:, h, 0:64],
                    start=st, stop=sp,
                )
                nc.tensor.matmul(
                    den[:, acc:acc + 1], est, ones_p[:],
                    start=st, stop=sp,
                )

        def emit_normalize(hg):
            # per-partition denominators: DVE reciprocal + tensor_scalar
            # multiply, then a DMA transpose into attnT (no PSUM, no PE).
            av, den = av_tiles[hg], den_tiles[hg]
            for qs in range(4):
                an = ascr_pool.tile([128, 128], F16, tag="an", name="an")
                for hh in range(2):
                    acc = 4 * hh + qs
                    rw = nrm_pool.tile([128, 1], F32, tag="rw", name="rw")
                    nc.vector.reciprocal(out=rw[:], in_=den[:, acc:acc + 1])
                    nc.vector.tensor_scalar_mul(
                        out=an[:, 64 * hh:64 * hh + 64],
                        in0=av[:, 64 * acc:64 * acc + 64],
                        scalar1=rw[:])
                # [128q, 128d-cat] -> attnT[hg][:, q-block] via DMA xbar
                nc.sync.dma_start_transpose(
                    out=attnT[hg][:, 128 * qs:128 * (qs + 1)], in_=an[:])

        for hg in range(9):
            for kc in range(KC):
                if hg >= 1 and kc % 2 == 1:
                    emit_av_acc(hg - 1, kc // 2)
                if hg < 8:
                    emit_scores(hg, kc)
            if hg == 5:
                wo_res = wo_res_pool.tile([128, 8, DIN], F16, tag="wores",
                                          name="wores")
                nc.sync.dma_start(out=wo_res[:], in_=wod.ap())
            if hg >= 1:
                emit_normalize(hg - 1)
            if dbg == "expt0" and hg == 1:
                dump([expt[(0, kc)] for kc in range(8)])
        p2.close()

        if dbg == "attnT":
            dump(attnT, w=TOK)

        # ---------------- phase 3: output projection ----------------
        for co in range(8):
            ps = scp.tile([128, TOK], F32, tag="sc", name="outps")
            for ci in range(8):
                nc.tensor.matmul(
                    ps[:], wo_res[:, ci, co * 128:(co + 1) * 128],
                    attnT[ci][:],
                    start=(ci == 0), stop=(ci == 7),
                )
            osb = wo_pool.tile([128, TOK], F32, tag="osb", name="osb")
            nc.vector.tensor_copy(osb[:], ps[:])
            nc.gpsimd.dma_start(out=out.ap()[co * 128:(co + 1) * 128, :],
                                in_=osb[:])

    nc.compile()
    return nc


def _host_prep(inputs):
    import ml_dtypes

    f16 = np.float16
    x = np.asarray(inputs["x"], np.float32)
    Wqkv = np.asarray(inputs["Wqkv"], np.float32)
    bqkv = np.asarray(inputs["bqkv"], np.float32)
    qs = np.asarray(inputs["q_scale"], np.float32)
    ks = np.asarray(inputs["k_scale"], np.float32)
    Wout = np.asarray(inputs["Wout"], np.float32)

    p64 = np.concatenate([np.arange(0, 64, 2), np.arange(1, 64, 2)])
    perm = np.concatenate([64 * h + p64 for h in range(H)])

    qsp, ksp = qs[perm], ks[perm]
    Wq = (Wqkv[:, :DIM] * qs[None, :])[:, perm]
    Wk = (Wqkv[:, DIM:2 * DIM] * ks[None, :])[:, perm]
    Wqk = np.concatenate([Wq, Wk], 1)                    # [1024, 2048]
    wqk = np.ascontiguousarray(
        Wqk.reshape(8, 128, 16, 128).transpose(2, 1, 0, 3)).astype(f16)
    wv = np.ascontiguousarray(
        Wqkv[:, 2 * DIM:].reshape(8, 128, DIM).transpose(1, 0, 2)).astype(f16)
    wo = np.ascontiguousarray(
        Wout.reshape(8, 128, DIN).transpose(1, 0, 2)).astype(f16)

    wsum = np.stack([1.0 / (DIM * qsp ** 2), 1.0 / (DIM * ksp ** 2)], 1)
    wsum = np.ascontiguousarray(
        wsum.reshape(8, 128, 2).transpose(1, 0, 2)).astype(f16)

    bq = (bqkv[:DIM] * qs)[perm]
    bk = (bqkv[DIM:2 * DIM] * ks)[perm]
    bias = np.concatenate([bq, bk, bqkv[2 * DIM:]])[None, :].astype(f16)

    sw = np.arange(128)
    swap = np.where(sw % 64 < 32, sw + 32, sw - 32)
    P = np.zeros((128, 128), np.float32)
    P[swap, np.arange(128)] = 1.0  # (P.T @ x)[m] = x[swap[m]]
    P = P.astype(f16)

    inv_freq = 1.0 / (BASE ** (np.arange(0, HD, 2).astype(np.float32) / HD))
    pos = np.maximum(np.arange(N) - 1, 0).astype(np.float32)
    ang = pos[:, None] * inv_freq[None, :]
    cosT, sinT = np.cos(ang).T, np.sin(ang).T           # (32, N)
    C128 = np.tile(cosT, (4, 1))                        # (128, N)
    S128 = np.concatenate([-sinT, sinT, -sinT, sinT], 0)

    in_maps = []
    for core in range(NCORE):
        b, sh = core // 4, core % 4
        t0 = sh * TOK
        xt = np.ascontiguousarray(
            x[b, t0:t0 + TOK, :].T.reshape(8, 128, TOK).transpose(1, 0, 2)
        ).astype(f16)
        tabsc = np.ascontiguousarray(
            np.stack([C128[:, t0:t0 + TOK], S128[:, t0:t0 + TOK]])
        ).astype(f16)
        in_maps.append({
            "xt": xt,
            "wqk": wqk,
            "wv": wv,
            "wo": wo,
            "bqkv": bias,
            "wsum": wsum,
            "swp": P,
            "tabs": tabsc,
        })
    return in_maps


LAST_EXEC_NS = None


def kernel(**inputs):
    global LAST_EXEC_NS
    import os
    from concourse.bass_utils import run_bass_kernel_spmd

    dbg = os.environ.get("KERNEL_DBG") or None
    zb = bool(np.all(np.asarray(inputs["bqkv"]) == 0))
    key = f"nc{dbg}{zb}"
    if key not in _CACHE:
        _CACHE[key] = _build_nc(dbg, zero_bias=zb)
    nc = _CACHE[key]

    in_maps = _host_prep(inputs)
    trace = bool(int(os.environ.get("KERNEL_TRACE", "0")))
    tmpdir = None
    if trace:
        import tempfile
        import concourse.bass_utils as _bu
        _bu.upload_artifacts = lambda d: d  # keep artifacts local
        tmpdir = tempfile.mkdtemp(prefix="ktrace_")
        print("TRACE DIR:", tmpdir)
    res = run_bass_kernel_spmd(
        nc, in_maps, core_ids=list(range(NCORE)), trace=trace, tmpdir=tmpdir
    )
    LAST_EXEC_NS = res.exec_time_ns
    bout = np.asarray(inputs["bout"], np.float32)
    out = np.empty((B, N, DIN), np.float32)
    for core in range(NCORE):
        b, sh = core // 4, core % 4
        t0 = sh * TOK
        out[b, t0:t0 + TOK, :] = res.results[core]["out"].T
    out += bout[None, None, :]
    return out


def kernel_raw(inputs):
    """Debug helper: run and return the per-core raw outputs."""
    global LAST_EXEC_NS
    import os
    from concourse.bass_utils import run_bass_kernel_spmd

    dbg = os.environ.get("KERNEL_DBG") or None
    zb = bool(np.all(np.asarray(inputs["bqkv"]) == 0))
    key = f"nc{dbg}{zb}"
    if key not in _CACHE:
        _CACHE[key] = _build_nc(dbg, zero_bias=zb)
    nc = _CACHE[key]
    in_maps = _host_prep(inputs)
    res = run_bass_kernel_spmd(nc, in_maps, core_ids=list(range(NCORE)))
    LAST_EXEC_NS = res.exec_time_ns
    key = "dbg" if dbg else "out"
    return [r[key] for r in res.results]
